# revision 1
# baseline (speedup 1.0000x reference)
"""CrossViewAttention Trainium2 Bass kernel.

Problem: q,kv [V=6,B=2,C=512,H=32,W=32]; per fused batch (12 total):
  kvp = kv_flat @ W_kv + b_kv -> k,v; 8-head attention(q, k, v); out = y @ W_proj + b_proj.

Layout trick: input (v b c h w) is ALREADY feature-major [C, T=H*W] per batch,
i.e. q^T / kv^T.  The whole pipeline runs in transposed space:
  k^T[c2,t]   = sum_c W_kv[c,c2] kv^T[c,t]          (lhsT=W_kv block, rhs=kv^T)
  v[t,d]      = sum_c kv^T[c,t] W_kv[c,512+d]        (lhsT=kv^T block, rhs=W_kv cols)
  S^T[k,q]    = sum_d kh^T[d,k] qh^T[d,q]            (lhsT=k^T slice, rhs=q^T slice)
  P^T         = exp(S^T/8)            (softmax over k = partition dim; no max-sub,
                                       scores bounded; denom via ones-column below)
  y'^T[d',q]  = sum_k [vh|1][k,d'] P^T[k,q]          (row 64 = softmax denominator)
  y^T         = y'^T[0:64] * recip(y'^T[64])
  out^T[c2,t] = sum_c W_proj[c,c2] y^T[c,t] + b_proj (native output layout)

Sharding (8 cores, no collectives): core c gets full batch c, plus half of the
query dim (half = c%2) of batch 8 + c//2 (kv-proj for split batches is
duplicated on both cores of a pair; attention/out-proj are halved).
"""

import numpy as np

V, BS = 6, 2
NB = V * BS          # 12 fused batches
C = 512
T = 1024
NH = 8
HD = 64
C2 = 2 * C
P = 128
NCORES = 8
TQ = 512             # q-chunk / matmul moving free dim

_CACHE = {}


def _build_nc(loop_n=0):
    from contextlib import ExitStack

    from concourse import bacc, mybir, tile

    F32 = mybir.dt.float32
    F32R = mybir.dt.float32r
    EXP = mybir.ActivationFunctionType.Exp
    ADD = mybir.AluOpType.add
    MUL = mybir.AluOpType.mult

    nc = bacc.Bacc("TRN2", target_bir_lowering=False, debug=False,
                   enable_asserts=True, num_devices=NCORES)

    qA = nc.dram_tensor("qA", [C, T], F32, kind="ExternalInput").ap()
    kvA = nc.dram_tensor("kvA", [C, T], F32, kind="ExternalInput").ap()
    qB = nc.dram_tensor("qB", [C, TQ], F32, kind="ExternalInput").ap()
    kvB = nc.dram_tensor("kvB", [C, T], F32, kind="ExternalInput").ap()
    wkv = nc.dram_tensor("wkv", [C, C2], F32, kind="ExternalInput").ap()
    bkv = nc.dram_tensor("bkv", [C2], F32, kind="ExternalInput").ap()
    wpr = nc.dram_tensor("wpr", [C, C], F32, kind="ExternalInput").ap()
    bpr = nc.dram_tensor("bpr", [C], F32, kind="ExternalInput").ap()
    outA = nc.dram_tensor("outA", [C, T], F32, kind="ExternalOutput").ap()
    outB = nc.dram_tensor("outB", [C, TQ], F32, kind="ExternalOutput").ap()

    from concourse.engine_type import EngineType

    with tile.TileContext(nc) as tc, ExitStack() as ctx:
        if loop_n:
            ctx.enter_context(tc.For_i(
                0, loop_n, 1,
                hint_engines=(EngineType.PE, EngineType.Activation,
                              EngineType.DVE, EngineType.Pool,
                              EngineType.SP)))
        consts = ctx.enter_context(tc.tile_pool(name="consts", bufs=1))
        qpool_a = ctx.enter_context(tc.tile_pool(name="qpa", bufs=1))
        qpool_b = ctx.enter_context(tc.tile_pool(name="qpb", bufs=1))
        kvpool = ctx.enter_context(tc.tile_pool(name="kvp", bufs=1))
        ktpool = ctx.enter_context(tc.tile_pool(name="ktp", bufs=2))
        vpool = ctx.enter_context(tc.tile_pool(name="vp", bufs=2))
        ptpool = ctx.enter_context(tc.tile_pool(name="ptp", bufs=6))
        ytpool_a = ctx.enter_context(tc.tile_pool(name="ytpa", bufs=1))
        ytpool_b = ctx.enter_context(tc.tile_pool(name="ytpb", bufs=1))
        rcpool = ctx.enter_context(tc.tile_pool(name="rcp", bufs=2))
        rbpool = ctx.enter_context(tc.tile_pool(name="rbp", bufs=2))
        outpool = ctx.enter_context(tc.tile_pool(name="op", bufs=3))
        psum_kv = ctx.enter_context(tc.tile_pool(name="pskv", bufs=2, space="PSUM"))
        psum_s = ctx.enter_context(tc.tile_pool(name="pss", bufs=2, space="PSUM"))
        psum_y = ctx.enter_context(tc.tile_pool(name="psy", bufs=2, space="PSUM"))

        # ---- constants + inputs; SWDGE is FIFO per queue, so issue in
        # need-order: wkv(k-half), kvA, wkv(v-half), bias-bcast, q, kvB, wpr.
        wkv_sb = consts.tile([P, 4, C2], F32R, tag="wkv")
        wkv_r = wkv.rearrange("(b p) n -> p b n", p=P)
        nc.gpsimd.dma_start(out=wkv_sb[:, :, 0:2 * P], in_=wkv_r[:, :, 0:2 * P])
        kva_sb = kvpool.tile([P, 4, T], F32R, tag="kv")
        kva_r = kvA.rearrange("(b p) t -> p b t", p=P)
        nc.gpsimd.dma_start(out=kva_sb[:, :, 0:TQ], in_=kva_r[:, :, 0:TQ])
        nc.gpsimd.dma_start(out=wkv_sb[:, :, 2 * P:C], in_=wkv_r[:, :, 2 * P:C])
        nc.gpsimd.dma_start(out=kva_sb[:, :, TQ:T], in_=kva_r[:, :, TQ:T])
        nc.gpsimd.dma_start(out=wkv_sb[:, :, C:C2], in_=wkv_r[:, :, C:C2])
        bv_bc = consts.tile([P, C], F32, tag="bv")
        nc.gpsimd.dma_start(out=bv_bc, in_=bkv[None, None, C:C2].broadcast_to([1, P, C]))
        qa_sb = qpool_a.tile([P, 4, T], F32R, tag="qa")
        nc.gpsimd.dma_start(out=qa_sb, in_=qA.rearrange("(b p) t -> p b t", p=P))
        qb_sb = qpool_b.tile([P, 4, TQ], F32R, tag="qb")
        nc.gpsimd.dma_start(out=qb_sb, in_=qB.rearrange("(b p) t -> p b t", p=P))
        kvb_sb = kvpool.tile([P, 4, T], F32R, tag="kv")
        nc.gpsimd.dma_start(out=kvb_sb, in_=kvB.rearrange("(b p) t -> p b t", p=P))
        wpr_sb = consts.tile([P, 4, C], F32R, tag="wpr")
        nc.gpsimd.dma_start(out=wpr_sb, in_=wpr.rearrange("(b p) n -> p b n", p=P))
        bk_sb = consts.tile([P, 4], F32, tag="bk")
        nc.sync.dma_start(out=bk_sb, in_=bkv[0:C].rearrange("(b p) -> p b", p=P))
        bp_sb = consts.tile([P, 4], F32, tag="bp")
        nc.sync.dma_start(out=bp_sb, in_=bpr.rearrange("(b p) -> p b", p=P))

        # ---- emission helpers ----
        def emit_kT_group(kv_sb, kT, c2b, tch):
            pk = psum_kv.tile([P, TQ], F32, tag="pkv")
            for cb in range(4):
                nc.tensor.matmul(
                    pk,
                    lhsT=wkv_sb[:, cb, c2b * P:(c2b + 1) * P],
                    rhs=kv_sb[:, cb, tch * TQ:(tch + 1) * TQ],
                    start=cb == 0, stop=cb == 3)
            nc.vector.tensor_tensor(
                out=kT[:, c2b, tch * TQ:(tch + 1) * TQ], in0=pk,
                in1=bk_sb[:, c2b:c2b + 1].broadcast_to([P, TQ]), op=ADD)

        def emit_v_ones(vsb):
            nc.vector.tensor_scalar(
                out=vsb[:, :, :, HD],
                in0=bv_bc[:, 0:HD].rearrange("p (a b) -> p a b", a=8),
                scalar1=0.0, scalar2=1.0, op0=MUL, op1=ADD)

        def emit_v_group(kv_sb, vsb, tb):
            pv = psum_kv.tile([P, C], F32, tag="pkv")
            for cb in range(4):
                nc.tensor.matmul(
                    pv,
                    lhsT=kv_sb[:, cb, tb * P:(tb + 1) * P],
                    rhs=wkv_sb[:, cb, C:C2],
                    start=cb == 0, stop=cb == 3)
            nc.vector.tensor_tensor(
                out=vsb[:, tb, :, 0:HD],
                in0=pv.rearrange("p (h d) -> p h d", h=NH),
                in1=bv_bc.rearrange("p (h d) -> p h d", h=NH), op=ADD)

        def emit_scores(q_sb, kT, h, qc):
            pb, po = h // 2, (h % 2) * HD
            pts = []
            for j in range(4):               # kb pairs
                s2 = psum_s.tile([P, 2, TQ], F32, tag="ps")
                for i in range(2):
                    kb = 2 * j + i
                    nc.tensor.matmul(
                        s2[:, i, :],
                        lhsT=kT[po:po + HD, pb, kb * P:(kb + 1) * P],
                        rhs=q_sb[po:po + HD, pb, qc * TQ:(qc + 1) * TQ],
                        start=True, stop=True)
                pt = ptpool.tile([P, 2, TQ], F32R, tag="pt")
                nc.scalar.activation(
                    out=pt.rearrange("p a b -> p (a b)"),
                    in_=s2.rearrange("p a b -> p (a b)"),
                    func=EXP, scale=0.125)
                pts.append(pt)
            return pts

        def emit_attv(vsb, yT, h, qc, pts):
            pb, po = h // 2, (h % 2) * HD
            py = psum_y.tile([HD + 1, TQ], F32, tag="py")
            for kb in range(8):
                nc.tensor.matmul(
                    py,
                    lhsT=vsb[:, kb, h, :],
                    rhs=pts[kb // 2][:, kb % 2, :],
                    start=kb == 0, stop=kb == 7)
            rc = rcpool.tile([1, TQ], F32, tag="rc")
            nc.vector.reciprocal(rc, py[HD:HD + 1, :])
            rb = rbpool.tile([HD, TQ], F32, tag="rb")
            nc.gpsimd.partition_broadcast(rb, rc)
            nc.vector.tensor_tensor(
                out=yT[po:po + HD, pb, qc * TQ:(qc + 1) * TQ],
                in0=py[0:HD, :], in1=rb, op=MUL)

        def emit_unit(q_sb, kT, vsb, yT, h, qc):
            pts = emit_scores(q_sb, kT, h, qc)
            emit_attv(vsb, yT, h, qc, pts)

        def outproj_start(yT, c2b, tch, ncb):
            """First ncb contraction steps of an out-proj group."""
            pk = psum_kv.tile([P, TQ], F32, tag="pkv")
            for cb in range(ncb):
                nc.tensor.matmul(
                    pk,
                    lhsT=wpr_sb[:, cb, c2b * P:(c2b + 1) * P],
                    rhs=yT[:, cb, tch * TQ:(tch + 1) * TQ],
                    start=cb == 0, stop=False)
            return pk

        def outproj_finish(pk, yT, out_dram, c2b, tch, cb0):
            for cb in range(cb0, 4):
                nc.tensor.matmul(
                    pk,
                    lhsT=wpr_sb[:, cb, c2b * P:(c2b + 1) * P],
                    rhs=yT[:, cb, tch * TQ:(tch + 1) * TQ],
                    start=False, stop=cb == 3)
            ot = outpool.tile([P, TQ], F32, tag="ot")
            nc.vector.tensor_tensor(
                out=ot, in0=pk,
                in1=bp_sb[:, c2b:c2b + 1].broadcast_to([P, TQ]), op=ADD)
            nc.sync.dma_start(
                out=out_dram[c2b * P:(c2b + 1) * P, tch * TQ:(tch + 1) * TQ],
                in_=ot)

        def emit_outproj_group(yT, out_dram, c2b, tch):
            pk = outproj_start(yT, c2b, tch, 4 - 1)
            outproj_finish(pk, yT, out_dram, c2b, tch, 3)

        # ---- batch A kv-proj; head-0 scores interleave with v-proj so the
        # scalar engine starts its exp stream ~15us earlier ----
        kT_A = ktpool.tile([P, 4, T], F32R, tag="kT")
        emit_kT_group(kva_sb, kT_A, 0, 0)
        emit_kT_group(kva_sb, kT_A, 0, 1)
        for c2b in range(1, 4):
            for tch in range(2):
                emit_kT_group(kva_sb, kT_A, c2b, tch)
        pts0 = emit_scores(qa_sb, kT_A, 0, 0)
        vsb_A = vpool.tile([P, 8, NH, HD + 1], F32R, tag="v")
        emit_v_ones(vsb_A)
        emit_v_group(kva_sb, vsb_A, 0)
        emit_v_group(kva_sb, vsb_A, 1)
        pts1 = emit_scores(qa_sb, kT_A, 1, 0)
        for tb in range(2, 8):
            emit_v_group(kva_sb, vsb_A, tb)

        # ---- batch B tiles (computed interleaved with batch A attention) ----
        kT_B = ktpool.tile([P, 4, T], F32R, tag="kT")
        vsb_B = vpool.tile([P, 8, NH, HD + 1], F32R, tag="v")
        emit_v_ones(vsb_B)
        fillers = [lambda c2b=c2b, tch=tch: emit_kT_group(kvb_sb, kT_B, c2b, tch)
                   for c2b in range(4) for tch in range(2)]
        fillers += [lambda tb=tb: emit_v_group(kvb_sb, vsb_B, tb) for tb in range(8)]

        yT_A = ytpool_a.tile([P, 4, 2 * TQ], F32R, tag="yT")
        yT_B = ytpool_b.tile([P, 4, TQ], F32R, tag="yT")

        # A attention qc0 (heads 0/1 scores already emitted above); last 4
        # units each preceded by one kvproj-B group
        emit_attv(vsb_A, yT_A, 0, 0, pts0)
        emit_attv(vsb_A, yT_A, 1, 0, pts1)
        for h in range(2, NH):
            if h >= 4:
                fillers.pop(0)()
            emit_unit(qa_sb, kT_A, vsb_A, yT_A, h, 0)
        # A attention qc1: drain remaining 12 kvproj-B groups, 1-2 per unit
        plan = (2, 1, 2, 1, 2, 1, 2, 1)
        for h in range(NH):
            for _ in range(plan[h]):
                if fillers:
                    fillers.pop(0)()
            emit_unit(qa_sb, kT_A, vsb_A, yT_A, h, 1)
        assert not fillers

        # B attention: outproj-A groups squeezed into units 0..5 so psum_kv
        # is free for the outproj-B split tail after unit 5
        opa = [(c2b, tch) for tch in range(2) for c2b in range(4)]
        plan_b = (2, 1, 2, 1, 1, 1, 0, 0)
        for h in range(NH):
            for _ in range(plan_b[h]):
                if opa:
                    c2b, tch = opa.pop(0)
                    emit_outproj_group(yT_A, outA, c2b, tch)
            emit_unit(qb_sb, kT_B, vsb_B, yT_B, h, 0)
            if h == 5:
                # out-proj B split tail: cb0-2 need only heads 0-5
                pk01 = [outproj_start(yT_B, c2b, 0, 3) for c2b in range(2)]
        for c2b in range(2):
            outproj_finish(pk01[c2b], yT_B, outB, c2b, 0, 3)
        for c2b in range(2, 4):
            emit_outproj_group(yT_B, outB, c2b, 0)

    nc.compile()
    return nc


def get_nc(loop_n=0):
    key = f"nc{loop_n}"
    if key not in _CACHE:
        _CACHE[key] = _build_nc(loop_n)
    return _CACHE[key]


def make_in_maps(q, kv, W_kv, b_kv, W_proj, b_proj):
    q = np.asarray(q, dtype=np.float32)
    kv = np.asarray(kv, dtype=np.float32)
    W_kv = np.asarray(W_kv, dtype=np.float32)
    b_kv = np.asarray(b_kv, dtype=np.float32)
    W_proj = np.asarray(W_proj, dtype=np.float32)
    b_proj = np.asarray(b_proj, dtype=np.float32)
    qf = np.ascontiguousarray(q.reshape(NB, C, T))
    kvf = np.ascontiguousarray(kv.reshape(NB, C, T))
    in_maps = []
    for c in range(NCORES):
        bA, bB, half = c, 8 + c // 2, c % 2
        in_maps.append({
            "qA": qf[bA],
            "kvA": kvf[bA],
            "qB": np.ascontiguousarray(qf[bB][:, half * TQ:(half + 1) * TQ]),
            "kvB": kvf[bB],
            "wkv": np.ascontiguousarray(W_kv),
            "bkv": np.ascontiguousarray(b_kv),
            "wpr": np.ascontiguousarray(W_proj),
            "bpr": np.ascontiguousarray(b_proj),
        })
    return in_maps


def assemble_out(results):
    out = np.empty((NB, C, T), np.float32)
    for c in range(NCORES):
        bB, half = 8 + c // 2, c % 2
        out[c] = results[c]["outA"]
        out[bB][:, half * TQ:(half + 1) * TQ] = results[c]["outB"]
    return out.reshape(V, BS, C, 32, 32)


def kernel(**inputs):
    from concourse.bass_utils import run_bass_kernel_spmd

    nc = get_nc()
    in_maps = make_in_maps(inputs["q"], inputs["kv"], inputs["W_kv"],
                           inputs["b_kv"], inputs["W_proj"], inputs["b_proj"])
    res = run_bass_kernel_spmd(nc, in_maps, core_ids=list(range(NCORES)))
    return assemble_out(res.results)



# revision 13
# speedup vs baseline: 1.5948x; 1.5948x over previous
"""CrossViewAttention Trainium2 Bass kernel.

Problem: q,kv [V=6,B=2,C=512,H=32,W=32]; per fused batch (12 total):
  kvp = kv_flat @ W_kv + b_kv -> k,v; 8-head attention(q, k, v); out = y @ W_proj + b_proj.

Layout trick: input (v b c h w) is ALREADY feature-major [C, T=H*W] per batch,
i.e. q^T / kv^T.  The whole pipeline runs in transposed space:
  k^T[c2,t]   = sum_c W_kv[c,c2] kv^T[c,t]          (lhsT=W_kv block, rhs=kv^T)
  v[t,d]      = sum_c kv^T[c,t] W_kv[c,512+d]        (lhsT=kv^T block, rhs=W_kv cols)
  S^T[k,q]    = sum_d kh^T[d,k] qh^T[d,q]            (lhsT=k^T slice, rhs=q^T slice)
  P^T         = exp(S^T/8)            (softmax over k = partition dim; no max-sub,
                                       scores bounded; denom via ones-column below)
  y'^T[d',q]  = sum_k [vh|1][k,d'] P^T[k,q]          (row 64 = softmax denominator)
  y^T         = y'^T[0:64] * recip(y'^T[64])
  out^T[c2,t] = sum_c W_proj[c,c2] y^T[c,t] + b_proj (native output layout)

Head-pair packing: score matmuls have K=64 (one head's d) — heads 2h/2h+1 sit
at partitions 0-63 / 64-127 of kT block h, so their score MMs carry
tile_position (0,0)/(64,0) and, issued back-to-back, stream CONCURRENTLY
through disjoint row-halves of the PE array (~2x score throughput on HW; the
cost-model sim charges them serially).  exp processes both heads' scores in
one [128, 2*512] activation; P and v are bf16 (attn is insensitive, SBUF win).

Software pipeline: the scalar engine (exp: 96 x ~1us) is the pacer.  Units are
head-pairs x q-chunk; in iteration i the PE emits unit i's score pairs
(feeding ACT), unit i-2's attv pairs (consuming its pts), plus filler matmul
groups (kv-proj / out-proj) scheduled to match DMA arrival order.

Sharding (8 cores, no collectives): core c gets full batch c, plus half of the
query dim (half = c%2) of batch 8 + c//2 (kv-proj for split batches is
duplicated on both cores of a pair; attention/out-proj are halved).
"""

import numpy as np

V, BS = 6, 2
NB = V * BS          # 12 fused batches
C = 512
T = 1024
NH = 8
HD = 64
C2 = 2 * C
P = 128
NCORES = 8
TQ = 512             # q-chunk / matmul moving free dim

_CACHE = {}


def _build_nc(loop_n=0):
    from contextlib import ExitStack

    from concourse import bacc, mybir, tile

    F32 = mybir.dt.float32
    F32R = mybir.dt.float32r
    BF16 = mybir.dt.bfloat16
    EXP = mybir.ActivationFunctionType.Exp
    ADD = mybir.AluOpType.add
    MUL = mybir.AluOpType.mult

    nc = bacc.Bacc("TRN2", target_bir_lowering=False, debug=False,
                   enable_asserts=True, num_devices=NCORES)

    qA = nc.dram_tensor("qA", [C, T], F32, kind="ExternalInput").ap()
    kvA = nc.dram_tensor("kvA", [C, T], F32, kind="ExternalInput").ap()
    qB = nc.dram_tensor("qB", [C, TQ], F32, kind="ExternalInput").ap()
    kvB = nc.dram_tensor("kvB", [C, T], F32, kind="ExternalInput").ap()
    wkv = nc.dram_tensor("wkv", [C, C2], F32, kind="ExternalInput").ap()
    bkv = nc.dram_tensor("bkv", [C2], F32, kind="ExternalInput").ap()
    wpr = nc.dram_tensor("wpr", [C, C], F32, kind="ExternalInput").ap()
    bpr = nc.dram_tensor("bpr", [C], F32, kind="ExternalInput").ap()
    outA = nc.dram_tensor("outA", [C, T], F32, kind="ExternalOutput").ap()
    outB = nc.dram_tensor("outB", [C, TQ], F32, kind="ExternalOutput").ap()

    from concourse.engine_type import EngineType

    with tile.TileContext(nc) as tc, ExitStack() as ctx:
        if loop_n:
            ctx.enter_context(tc.For_i(
                0, loop_n, 1,
                hint_engines=(EngineType.PE, EngineType.Activation,
                              EngineType.DVE, EngineType.Pool,
                              EngineType.SP)))
        consts = ctx.enter_context(tc.tile_pool(name="consts", bufs=1))
        qpool_a = ctx.enter_context(tc.tile_pool(name="qpa", bufs=1))
        qpool_b = ctx.enter_context(tc.tile_pool(name="qpb", bufs=1))
        kvpool = ctx.enter_context(tc.tile_pool(name="kvp", bufs=1))
        ktpool = ctx.enter_context(tc.tile_pool(name="ktp", bufs=2))
        vpool = ctx.enter_context(tc.tile_pool(name="vp", bufs=2))
        ptpool = ctx.enter_context(tc.tile_pool(name="ptp", bufs=18))
        ytpool_a = ctx.enter_context(tc.tile_pool(name="ytpa", bufs=1))
        ytpool_b = ctx.enter_context(tc.tile_pool(name="ytpb", bufs=1))
        rcpool = ctx.enter_context(tc.tile_pool(name="rcp", bufs=2))
        rbpool = ctx.enter_context(tc.tile_pool(name="rbp", bufs=2))
        outpool = ctx.enter_context(tc.tile_pool(name="op", bufs=3))
        psum_kv = ctx.enter_context(tc.tile_pool(name="pskv", bufs=2, space="PSUM"))
        psum_s = ctx.enter_context(tc.tile_pool(name="pss", bufs=2, space="PSUM"))
        psum_y = ctx.enter_context(tc.tile_pool(name="psy", bufs=2, space="PSUM"))

        # ---- inputs; SWDGE is FIFO per queue -> strict need-order:
        # kT_A block0 (wkv k-lo + kvA) -> qa head-pair 0 -> v-bias bcast ->
        # wkv k-hi -> qa rest -> wkv v-half -> kvB -> qB -> W_proj.
        wkv_sb = consts.tile([P, 4, C2], F32R, tag="wkv")
        wkv_r = wkv.rearrange("(b p) n -> p b n", p=P)
        nc.gpsimd.dma_start(out=wkv_sb[:, :, 0:2 * P], in_=wkv_r[:, :, 0:2 * P])
        kva_sb = kvpool.tile([P, 4, T], F32R, tag="kv")
        kva_r = kvA.rearrange("(b p) t -> p b t", p=P)
        nc.gpsimd.dma_start(out=kva_sb[:, :, 0:TQ], in_=kva_r[:, :, 0:TQ])
        qa_sb = qpool_a.tile([P, 4, T], F32R, tag="qa")
        qa_r = qA.rearrange("(b p) t -> p b t", p=P)
        nc.gpsimd.dma_start(out=qa_sb[:, 0:1, :], in_=qa_r[:, 0:1, :])
        nc.gpsimd.dma_start(out=kva_sb[:, :, TQ:T], in_=kva_r[:, :, TQ:T])
        bv_bc = consts.tile([P, C], F32, tag="bv")
        nc.gpsimd.dma_start(out=bv_bc, in_=bkv[None, None, C:C2].broadcast_to([1, P, C]))
        nc.gpsimd.dma_start(out=wkv_sb[:, :, 2 * P:C], in_=wkv_r[:, :, 2 * P:C])
        nc.gpsimd.dma_start(out=qa_sb[:, 1:2, :], in_=qa_r[:, 1:2, :])
        nc.gpsimd.dma_start(out=qa_sb[:, 2:4, :], in_=qa_r[:, 2:4, :])
        nc.gpsimd.dma_start(out=wkv_sb[:, :, C:C2], in_=wkv_r[:, :, C:C2])
        kvb_sb = kvpool.tile([P, 4, T], F32R, tag="kv")
        nc.gpsimd.dma_start(out=kvb_sb, in_=kvB.rearrange("(b p) t -> p b t", p=P))
        qb_sb = qpool_b.tile([P, 4, TQ], F32R, tag="qb")
        nc.gpsimd.dma_start(out=qb_sb, in_=qB.rearrange("(b p) t -> p b t", p=P))
        wpr_sb = consts.tile([P, 4, C], F32R, tag="wpr")
        nc.gpsimd.dma_start(out=wpr_sb, in_=wpr.rearrange("(b p) n -> p b n", p=P))
        bk_sb = consts.tile([P, 4], F32, tag="bk")
        nc.sync.dma_start(out=bk_sb, in_=bkv[0:C].rearrange("(b p) -> p b", p=P))
        bp_sb = consts.tile([P, 4], F32, tag="bp")
        nc.sync.dma_start(out=bp_sb, in_=bpr.rearrange("(b p) -> p b", p=P))

        # ---- emission helpers ----
        def emit_kT_group(kv_sb, kT, c2b, tch):
            pk = psum_kv.tile([P, TQ], F32, tag="pkv")
            for cb in range(4):
                nc.tensor.matmul(
                    pk,
                    lhsT=wkv_sb[:, cb, c2b * P:(c2b + 1) * P],
                    rhs=kv_sb[:, cb, tch * TQ:(tch + 1) * TQ],
                    start=cb == 0, stop=cb == 3)
            nc.vector.tensor_tensor(
                out=kT[:, c2b, tch * TQ:(tch + 1) * TQ], in0=pk,
                in1=bk_sb[:, c2b:c2b + 1].broadcast_to([P, TQ]), op=ADD)

        def emit_v_ones(vsb):
            nc.vector.tensor_scalar(
                out=vsb[:, :, :, HD],
                in0=bv_bc[:, 0:HD].rearrange("p (a b) -> p a b", a=8),
                scalar1=0.0, scalar2=1.0, op0=MUL, op1=ADD)

        def emit_v_group(kv_sb, vsb, tb):
            pv = psum_kv.tile([P, C], F32, tag="pkv")
            for cb in range(4):
                nc.tensor.matmul(
                    pv,
                    lhsT=kv_sb[:, cb, tb * P:(tb + 1) * P],
                    rhs=wkv_sb[:, cb, C:C2],
                    start=cb == 0, stop=cb == 3)
            nc.vector.tensor_tensor(
                out=vsb[:, tb, :, 0:HD],
                in0=pv.rearrange("p (h d) -> p h d", h=NH),
                in1=bv_bc.rearrange("p (h d) -> p h d", h=NH), op=ADD)

        def emit_score_pair(q_sb, kT, hp, qc, kb):
            """Row-packed scores for heads (2hp, 2hp+1), one kb block.
            Returns the [P, 2, TQ] bf16 prob tile (slot e = head 2hp+e)."""
            s2 = psum_s.tile([P, 2, TQ], F32, tag="ps")
            for e in range(2):
                po = e * HD
                nc.tensor.matmul(
                    s2[:, e, :],
                    lhsT=kT[po:po + HD, hp, kb * P:(kb + 1) * P],
                    rhs=q_sb[po:po + HD, hp, qc * TQ:(qc + 1) * TQ],
                    start=True, stop=True)
            pt = ptpool.tile([P, 2, TQ], BF16, tag="pt")
            nc.scalar.activation(
                out=pt.rearrange("p a b -> p (a b)"),
                in_=s2.rearrange("p a b -> p (a b)"),
                func=EXP, scale=0.125)
            return pt

        def emit_attv_pair(rec, pys, kb):
            """attv for heads (2hp, 2hp+1) of a pending unit, one kb block."""
            pts, vsb, _, hp, _ = rec
            for e in range(2):
                nc.tensor.matmul(
                    pys[e],
                    lhsT=vsb[:, kb, 2 * hp + e, :],
                    rhs=pts[kb][:, e, :],
                    start=kb == 0, stop=kb == 7)

        def emit_norm(rec, pys):
            """Normalize both heads of a finished attv pair into yT."""
            _, _, yT, hp, qc = rec
            for e in range(2):
                po = e * HD
                py = pys[e]
                rc = rcpool.tile([1, TQ], F32, tag="rc")
                nc.vector.reciprocal(rc, py[HD:HD + 1, :])
                rb = rbpool.tile([HD, TQ], F32, tag="rb")
                nc.gpsimd.partition_broadcast(rb, rc)
                nc.vector.tensor_tensor(
                    out=yT[po:po + HD, hp, qc * TQ:(qc + 1) * TQ],
                    in0=py[0:HD, :], in1=rb, op=MUL)

        def outproj_start(yT, c2b, tch, ncb, pool=None):
            """First ncb contraction steps of an out-proj group."""
            pk = (pool or psum_kv).tile([P, TQ], F32, tag="pkv", name="pk")
            for cb in range(ncb):
                nc.tensor.matmul(
                    pk,
                    lhsT=wpr_sb[:, cb, c2b * P:(c2b + 1) * P],
                    rhs=yT[:, cb, tch * TQ:(tch + 1) * TQ],
                    start=cb == 0, stop=False)
            return pk

        def outproj_finish(pk, yT, out_dram, c2b, tch, cb0):
            for cb in range(cb0, 4):
                nc.tensor.matmul(
                    pk,
                    lhsT=wpr_sb[:, cb, c2b * P:(c2b + 1) * P],
                    rhs=yT[:, cb, tch * TQ:(tch + 1) * TQ],
                    start=False, stop=cb == 3)
            ot = outpool.tile([P, TQ], F32, tag="ot")
            nc.vector.tensor_tensor(
                out=ot, in0=pk,
                in1=bp_sb[:, c2b:c2b + 1].broadcast_to([P, TQ]), op=ADD)
            nc.sync.dma_start(
                out=out_dram[c2b * P:(c2b + 1) * P, tch * TQ:(tch + 1) * TQ],
                in_=ot)

        def emit_outproj_group(yT, out_dram, c2b, tch):
            pk = outproj_start(yT, c2b, tch, 4 - 1)
            outproj_finish(pk, yT, out_dram, c2b, tch, 3)

        # ---- prep tiles ----
        kT_A = ktpool.tile([P, 4, T], F32R, tag="kT")
        kT_B = ktpool.tile([P, 4, T], F32R, tag="kT")
        vsb_A = vpool.tile([P, 8, NH, HD + 1], BF16, tag="v")
        vsb_B = vpool.tile([P, 8, NH, HD + 1], BF16, tag="v")
        yT_A = ytpool_a.tile([P, 4, 2 * TQ], F32R, tag="yT")
        yT_B = ytpool_b.tile([P, 4, TQ], F32R, tag="yT")

        # Unit stream: (q_sb, kT, vsb, yT, hp, qc), A (qc0 then qc1) then B.
        units = [(qa_sb, kT_A, vsb_A, yT_A, hp, qc)
                 for qc in range(2) for hp in range(4)]
        units += [(qb_sb, kT_B, vsb_B, yT_B, hp, 0) for hp in range(4)]

        # Fillers, in DMA-arrival / need order; slots[i] gives the kb slot at
        # which each of iteration i's fillers is popped (spread to keep PE fed
        # without bulges, and to land v-groups ahead of the attv that reads
        # them).
        fillers = [lambda c2b=c2b, tch=tch: emit_kT_group(kva_sb, kT_A, c2b, tch)
                   for c2b in range(1, 4) for tch in range(2)]
        fillers += [lambda: emit_v_ones(vsb_A)]
        fillers += [lambda tb=tb: emit_v_group(kva_sb, vsb_A, tb) for tb in range(8)]
        fillers += [lambda c2b=c2b, tch=tch: emit_kT_group(kvb_sb, kT_B, c2b, tch)
                    for c2b in range(4) for tch in range(2)]
        fillers += [lambda: emit_v_ones(vsb_B)]
        fillers += [lambda tb=tb: emit_v_group(kvb_sb, vsb_B, tb) for tb in range(8)]
        fillers += [lambda c2b=c2b, tch=tch: emit_outproj_group(yT_A, outA, c2b, tch)
                    for tch in range(2) for c2b in range(4)]
        slots = (
            (0, 2, 4, 6),                # kT_A (1,0) (1,1) (2,0) (2,1)
            (0, 1, 3, 4, 5, 6, 7),       # kT_A (3,0) (3,1), v_ones, v0-v3
            (0, 1, 2, 3),                # v4-v7
            (0, 1, 4, 5),                # kT_B (0,0) (0,1) (1,0) (1,1)
            (0, 2, 4, 6),                # kT_B (2,0) (2,1) (3,0) (3,1)
            (0, 3, 6),                   # v_onesB, vB0, vB1
            (0, 3, 6),                   # vB2-vB4
            (0, 3, 6),                   # vB5-vB7
            (0, 4), (0, 4), (0, 4), (0, 4),   # outproj_A pairs
        )
        assert sum(len(s) for s in slots) == len(fillers)

        # ---- software pipeline (lag 2: attv of unit i runs in iteration i+2)
        emit_kT_group(kva_sb, kT_A, 0, 0)
        emit_kT_group(kva_sb, kT_A, 0, 1)

        pend = []           # pending unit records: (pts, vsb, yT, hp, qc)
        pk01 = None

        def run_iteration(unit, sl, fill, pys_pool=None):
            """One pipeline iteration: scores for `unit` (or None when
            draining), fillers at their kb slots, attv+norm for the unit two
            back."""
            rec = pend.pop(0) if (unit is None or len(pend) == 2) else None
            pys = None
            if rec is not None:
                pool = pys_pool or psum_y
                py_e = pool.tile([HD + 1, TQ], F32, tag="py", name="py_e")
                py_o = pool.tile([HD + 1, TQ], F32, tag="py", name="py_o")
                pys = [py_e, py_o]
            pts = []
            for kb in range(8):
                if unit is not None:
                    q_sb, kT, _, _, hp, qc = unit
                    pts.append(emit_score_pair(q_sb, kT, hp, qc, kb))
                for _ in range(sl.count(kb)):
                    fill.pop(0)()
                if rec is not None:
                    emit_attv_pair(rec, pys, kb)
            if rec is not None:
                emit_norm(rec, pys)
            if unit is not None:
                _, _, vsb, yT, hp, qc = unit
                pend.append((pts, vsb, yT, hp, qc))

        for ui, unit in enumerate(units):
            run_iteration(unit, slots[ui], fillers)
        assert not fillers
        # Drain.  out-proj B c2b0/1 start early with the cb0/1 accumulation
        # (pairs 0/1 normalized long ago); cb2/cb3 join in the finish once
        # norms of units 10/11 land.
        pk01 = [outproj_start(yT_B, c2b, 0, 2) for c2b in range(2)]
        run_iteration(None, (), fillers)          # attv + norm, unit 10
        run_iteration(None, (), fillers)          # attv + norm, unit 11
        for c2b in range(2):
            outproj_finish(pk01[c2b], yT_B, outB, c2b, 0, 2)
        for c2b in range(2, 4):
            emit_outproj_group(yT_B, outB, c2b, 0)

    nc.compile()
    return nc


def get_nc(loop_n=0):
    key = f"nc{loop_n}"
    if key not in _CACHE:
        _CACHE[key] = _build_nc(loop_n)
    return _CACHE[key]


def make_in_maps(q, kv, W_kv, b_kv, W_proj, b_proj):
    q = np.asarray(q, dtype=np.float32)
    kv = np.asarray(kv, dtype=np.float32)
    W_kv = np.asarray(W_kv, dtype=np.float32)
    b_kv = np.asarray(b_kv, dtype=np.float32)
    W_proj = np.asarray(W_proj, dtype=np.float32)
    b_proj = np.asarray(b_proj, dtype=np.float32)
    qf = np.ascontiguousarray(q.reshape(NB, C, T))
    kvf = np.ascontiguousarray(kv.reshape(NB, C, T))
    in_maps = []
    for c in range(NCORES):
        bA, bB, half = c, 8 + c // 2, c % 2
        in_maps.append({
            "qA": qf[bA],
            "kvA": kvf[bA],
            "qB": np.ascontiguousarray(qf[bB][:, half * TQ:(half + 1) * TQ]),
            "kvB": kvf[bB],
            "wkv": np.ascontiguousarray(W_kv),
            "bkv": np.ascontiguousarray(b_kv),
            "wpr": np.ascontiguousarray(W_proj),
            "bpr": np.ascontiguousarray(b_proj),
        })
    return in_maps


def assemble_out(results):
    out = np.empty((NB, C, T), np.float32)
    for c in range(NCORES):
        bB, half = 8 + c // 2, c % 2
        out[c] = results[c]["outA"]
        out[bB][:, half * TQ:(half + 1) * TQ] = results[c]["outB"]
    return out.reshape(V, BS, C, 32, 32)


def kernel(**inputs):
    from concourse.bass_utils import run_bass_kernel_spmd

    nc = get_nc()
    in_maps = make_in_maps(inputs["q"], inputs["kv"], inputs["W_kv"],
                           inputs["b_kv"], inputs["W_proj"], inputs["b_proj"])
    res = run_bass_kernel_spmd(nc, in_maps, core_ids=list(range(NCORES)))
    return assemble_out(res.results)
